# revision 19
# baseline (speedup 1.0000x reference)
"""Trainium2 Bass kernel for nn_Classifier_36618891166176 (R-GCN message passing).

Strategy (8 NeuronCores, SPMD single program):
  - Partition dst nodes across cores (12500 each). Each core processes the
    edges whose dst lies in its range, for all 4 relations and all 3 layers.
  - Per layer, per (window-pair, src-chunk): ONE dma_gather call fetches the
    h[src] rows for all edges of both windows and all 4 relations (edges
    sorted by (window, relation), trailing -1 indices are stripped by the Q7
    ucode so each core only pays descriptors for its true edge count).
  - The one-hot aggregation matrices (sel) are generated ON-CHIP on the idle
    vector engine: sel = is_equal(IOTA, slot) * ce, two batched tensor_tensor
    passes with stride-0 broadcast of per-column slot/ce vectors. Rows not
    belonging to a (window, relation) piece have slot=-1 -> sel row 0, which
    also masks stale gather rows. This removes the 72MB/layer sel stream of
    the previous version entirely.
  - Aggregation per piece (tile, window, relation): matmul(psa[w][r],
    lhsT=x_tile, rhs=sel_col) accumulated in PSUM; then per window the
    per-relation weight matmuls, relu, store; AllGather into shared h.
  - AvgPool via host-precomputed pool-sel matmul + AllReduce, as before.

dma_gather indices are int16 (max 32768 rows), so h is addressed in 4 chunks
of 25000 rows. The index tile only needs the 32-partition band of the queue
that serves the call (Q7 cores 2q, 2q+1), so the host index slab holds 2
replicas instead of 8.
"""
import os
import sys
import numpy as np

import concourse.bacc as bacc
import concourse.bass as bass
import concourse.mybir as mybir
import concourse.tile as tile
from concourse.bass_utils import run_bass_kernel_spmd
from concourse import library_config

# Problem constants (hardcoded per harness contract).
N = 100000
E = 1000000
R = 4
G = 256
D = 128          # feature dim (DIN == DH == 128)
NCLS = 10
L = 3
NCORES = 8
NSLICE = N // NCORES          # 12500 dst nodes per core
NW = (NSLICE + 127) // 128    # 98 dst windows per core
LAST_W_ROWS = NSLICE - (NW - 1) * 128  # 84
NP_ = NW // 2                 # 49 window pairs
CHUNK = 25000
NCHUNK = (N + CHUNK - 1) // CHUNK      # 4
P = 128

_CACHE = {}

# bisection knobs (default = full-featured kernel)
_KLAYERS = int(os.environ.get("KLAYERS", "3"))
_KBAND = int(os.environ.get("KBAND", "1"))    # 1: 32-part band idx, 0: 8-replica
# NOTE: trailing -1 stripping (KSTRIP=1) hangs the SWDGE ring: the decode
# side reserves ring space from the num_idxs register while the Q7 kernel
# writes descriptors per the value-stripped count; the bookkeeping drift
# wedges the queue after enough calls. Zero-padding is mandatory.
_KSTRIP = int(os.environ.get("KSTRIP", "0"))  # 1: -1 pad (stripped), 0: idx-0 pad
_KSCRATCH = int(os.environ.get("KSCRATCH", "65536"))


def _prep(edges, graph_ids):
    """Host-side edge preprocessing. Layer-invariant. Returns per-core arrays
    plus the uniform schedule."""
    import ml_dtypes
    bf = ml_dtypes.bfloat16
    edges = np.asarray(edges)
    graph_ids = np.asarray(graph_ids)

    # Flatten all relations' edges with per-edge normalization coefficient.
    srcs = []
    dsts = []
    ces = []
    rels = []
    for r in range(R):
        src = edges[r, 0].astype(np.int64)
        dst = edges[r, 1].astype(np.int64)
        deg_out = np.maximum(np.bincount(src, minlength=N), 1.0)
        deg_in = np.maximum(np.bincount(dst, minlength=N), 1.0)
        ce = (1.0 / np.sqrt(deg_out[src]) / np.sqrt(deg_in[dst])).astype(np.float32)
        srcs.append(src)
        dsts.append(dst)
        ces.append(ce)
        rels.append(np.full(E, r, dtype=np.int64))
    src = np.concatenate(srcs)
    dst = np.concatenate(dsts)
    ce = np.concatenate(ces)
    rel = np.concatenate(rels)

    core = dst // NSLICE
    local = dst - core * NSLICE
    w = local >> 7
    slot = local & 127
    pair = w >> 1
    wl = w & 1
    ch = src // CHUNK
    srcl = (src % CHUNK).astype(np.int64)

    # Sort edges by (core, pair, ch, wl, rel).
    key = ((((core * NP_ + pair) * NCHUNK + ch) * 2 + wl) * R + rel)
    order = np.argsort(key, kind="stable")
    key = key[order]
    srcl = srcl[order]
    slot = slot[order]
    ce = ce[order]

    NSEG = NCORES * NP_ * NCHUNK          # (core, pair, ch) segments
    seg_key = key // (2 * R)
    seg_bounds = np.searchsorted(seg_key, np.arange(NSEG + 1))
    seg_counts = np.diff(seg_bounds)      # [NSEG]
    seg_start = seg_bounds[:-1]
    # rank of each edge within its (core, pair, ch) segment
    k_local = np.arange(len(key)) - np.repeat(seg_start, seg_counts)

    # nt per (pair, ch) = ceil(max over cores / 128)
    cnt = seg_counts.reshape(NCORES, NP_, NCHUNK)
    nt = np.maximum(1, -(-cnt.max(axis=0) // 128))   # [NP_, NCHUNK]
    flat = nt.reshape(-1)
    tb = np.concatenate([[0], np.cumsum(flat)[:-1]])
    tile_base = tb.reshape(NP_, NCHUNK)
    TT = int(flat.sum())
    EPAD = TT * P

    # Subcell bounds per (core, pair, ch, wl, r) for piece enumeration.
    NSUB = NSEG * 2 * R
    sub_bounds = np.searchsorted(key, np.arange(NSUB + 1))
    sub_a = sub_bounds[:-1]
    sub_b = sub_bounds[1:]
    # local (within segment) bounds
    seg_of_sub = np.arange(NSUB) // (2 * R)
    la = sub_a - seg_start[seg_of_sub]
    lb = sub_b - seg_start[seg_of_sub]

    # Pieces per (pair, ch): union over cores of (wl, r, t) tile spans.
    # coltab[(pair, ch, wl, r), t] -> global column id or -1
    pieces = {}   # (pair, ch) -> list of (wl, r, t)
    for pch in range(NP_ * NCHUNK):
        pieces[pch] = set()
    sub_ids = np.arange(NSUB)
    core_of_sub = sub_ids // (NP_ * NCHUNK * 2 * R)
    rem = sub_ids % (NP_ * NCHUNK * 2 * R)
    pch_of_sub = rem // (2 * R)
    wlr_of_sub = rem % (2 * R)
    nz = lb > la
    for i in np.nonzero(nz)[0]:
        t0 = la[i] // P
        t1 = (lb[i] - 1) // P
        s = pieces[pch_of_sub[i]]
        wlr = wlr_of_sub[i]
        for t in range(t0, t1 + 1):
            s.add((wlr, t))

    # Assign global column ids, ordered by (pair, ch, wl, r, t).
    col_of = {}       # (pch, wlr, t) -> global col
    cols_per_pch = []  # pch -> ordered piece list [(wlr, t, col)]
    colbase_pch = np.zeros(NP_ * NCHUNK + 1, dtype=np.int64)
    cid = 0
    for pch in range(NP_ * NCHUNK):
        colbase_pch[pch] = cid
        lst = sorted(pieces[pch])
        out = []
        for (wlr, t) in lst:
            col_of[(pch, wlr, t)] = cid
            out.append((wlr, t, cid))
            cid += 1
        cols_per_pch.append(out)
    colbase_pch[NP_ * NCHUNK] = cid
    TOTCOL = cid

    # Per-edge column id for the slot/ce scatter.
    # edge -> (pch, wlr, t_e) where t_e = k_local // 128
    pch_e = seg_key % (NP_ * NCHUNK)      # (pair, ch), core stripped
    wlr_e = key % (2 * R)
    t_e = k_local // P
    # build lookup array: max tiles per pch
    NTMAXG = int(nt.max())
    lut = np.full((NP_ * NCHUNK * 2 * R, NTMAXG + 2), -1, dtype=np.int64)
    for (pch, wlr, t), c in col_of.items():
        lut[pch * 2 * R + wlr, t] = c
    colidx_e = lut[pch_e * 2 * R + wlr_e, t_e]
    assert colidx_e.min() >= 0

    # Per-core arrays.
    core_e = seg_key // (NP_ * NCHUNK)
    part_e = k_local % P

    slotslab = np.full((NCORES, P, TOTCOL), -1.0, dtype=bf)
    ceslab = np.zeros((NCORES, P, TOTCOL), dtype=bf)
    slotslab[core_e, part_e, colidx_e] = slot.astype(bf)
    ceslab[core_e, part_e, colidx_e] = ce.astype(bf)

    # Gather index slab: [NCORES, 32, EPAD/16] int16, init -1 (trailing strip).
    # Index k of call (pair, ch) sits at (16*rep + k%16, tile_base*8 + k//16),
    # rep in {0, 1} (only the call's queue band is ever read; the DMA writes
    # the band for queue q = (pair*NCHUNK+ch) % 4).
    nrep = 2 if _KBAND else 8
    fillv = -1 if _KSTRIP else 0
    gidx = np.full((NCORES, 16 * nrep, EPAD // 16), fillv, dtype=np.int16)
    tb_e = tile_base[(pch_e // NCHUNK), (pch_e % NCHUNK)]
    gcol = tb_e * 8 + k_local // 16
    grow = (k_local % 16).astype(np.int64)
    s16 = srcl.astype(np.int16)
    for rep in range(nrep):
        gidx[core_e, grow + 16 * rep, gcol] = s16

    # Pool sel slabs: [128, NW, 256] bf16 per core (1/count at graph id).
    counts = np.maximum(np.bincount(graph_ids.astype(np.int64), minlength=G),
                        1.0).astype(np.float32)
    psel = np.zeros((NCORES, P, NW, G), dtype=bf)
    for c in range(NCORES):
        g = graph_ids[c * NSLICE:(c + 1) * NSLICE].astype(np.int64)
        nodes = np.arange(NSLICE)
        wv = nodes >> 7
        pv = nodes & 127
        psel[c, pv, wv, g] = (1.0 / counts[g]).astype(bf)

    # max columns within one (pair, ch) chunk-slab
    CMAXC = int(np.diff(colbase_pch).max())

    sched = {
        "nt": nt, "tile_base": tile_base, "TT": TT, "EPAD": EPAD,
        "cols_per_pch": cols_per_pch, "colbase_pch": colbase_pch,
        "TOTCOL": TOTCOL, "CMAXC": CMAXC, "NTMAXG": NTMAXG,
    }
    # IOTA constant [128, CMAXC*128] bf16: per partition, 0..127 repeated.
    iota = np.tile(np.arange(P, dtype=np.float32), CMAXC)
    iota = np.broadcast_to(iota, (P, CMAXC * P)).astype(bf)
    arrays = {
        "gidx": gidx, "slotslab": slotslab, "ceslab": ceslab, "psel": psel,
        "iota": np.ascontiguousarray(iota),
    }
    return sched, arrays


def _build(sched):
    """Build the SPMD bass program for the uniform schedule."""
    nt = sched["nt"]
    tile_base = sched["tile_base"]
    EPAD = sched["EPAD"]
    cols_per_pch = sched["cols_per_pch"]
    colbase_pch = sched["colbase_pch"]
    TOTCOL = sched["TOTCOL"]
    CMAXC = sched["CMAXC"]
    NTMAXG = sched["NTMAXG"]

    f32 = mybir.dt.float32
    bf16 = mybir.dt.bfloat16
    i16 = mybir.dt.int16

    nc = bacc.Bacc("TRN2", target_bir_lowering=False, debug=False,
                   num_swdge_queues=4, dynamic_dma_scratch_size=_KSCRATCH)

    feat = nc.dram_tensor("feat", [N, D], bf16, kind="ExternalInput")
    wstack = nc.dram_tensor("wstack", [L * R, D, D], bf16, kind="ExternalInput")
    wc = nc.dram_tensor("wc", [D, NCLS], f32, kind="ExternalInput")
    bcb = nc.dram_tensor("bcb", [NCLS, 1], f32, kind="ExternalInput")
    IDXROWS = 32 if _KBAND else P
    gidx_d = nc.dram_tensor("gidx", [IDXROWS, EPAD // 16], i16,
                            kind="ExternalInput")
    slot_d = nc.dram_tensor("slotslab", [P, TOTCOL], bf16, kind="ExternalInput")
    ce_d = nc.dram_tensor("ceslab", [P, TOTCOL], bf16, kind="ExternalInput")
    iota_d = nc.dram_tensor("iota", [P, CMAXC * P], bf16, kind="ExternalInput")
    psel_d = nc.dram_tensor("psel", [P, NW * G], bf16, kind="ExternalInput")

    h_full = [None,
              nc.dram_tensor("h1f", [N, D], bf16, addr_space="Shared"),
              nc.dram_tensor("h2f", [N, D], bf16, addr_space="Shared")]
    h_slice = [nc.dram_tensor(f"hs{l}", [NSLICE, D], bf16) for l in range(L)]
    poolin = nc.dram_tensor("poolin", [P, G], f32)
    poolout = nc.dram_tensor("poolout", [P, G], f32, addr_space="Shared")
    out_d = nc.dram_tensor("out", [NCLS, G], f32, kind="ExternalOutput")

    cc_sem = nc.alloc_semaphore("ccsem")
    cc_count = [0]

    with tile.TileContext(nc) as tc:
        nc.gpsimd.load_library(library_config.mlp)

        with tc.tile_pool(name="const", bufs=1) as cpool:
            w_sb = []
            for i in range(L * R):
                t = cpool.tile([D, D], bf16, tag=f"w{i}")
                nc.sync.dma_start(t[:], wstack[i])
                w_sb.append(t)
            wc_sb = cpool.tile([D, NCLS], f32, tag="wc")
            nc.sync.dma_start(wc_sb[:], wc[:])
            bc_sb = cpool.tile([NCLS, 1], f32, tag="bc")
            nc.sync.dma_start(bc_sb[:], bcb[:])
            iota_sb = cpool.tile([P, CMAXC, P], bf16, tag="iota")
            nc.sync.dma_start(iota_sb[:], iota_d[:])

            nidx_regs = {}
            for p_ in range(NP_):
                for c_ in range(NCHUNK):
                    v = int(nt[p_, c_]) * P
                    if v not in nidx_regs:
                        nidx_regs[v] = nc.gpsimd.to_reg(v)

            XBUFS = 8
            with tc.tile_pool(name="xg", bufs=XBUFS) as xp_g:
                # memset the x pool once: stale (unwritten) rows are fed to
                # the PE multiplied by sel=0; they must never be NaN bits.
                for _ in range(XBUFS):
                    t = xp_g.tile([P, NTMAXG, D], bf16, tag="x")
                    nc.vector.memset(t[:], 0.0)

                def run_layer(layer, table, out_slice):
                    with tc.tile_pool(name=f"idx{layer}", bufs=4) as idxp, \
                         tc.tile_pool(name=f"sc{layer}", bufs=4) as scp, \
                         tc.tile_pool(name=f"sel{layer}", bufs=6) as selp, \
                         tc.tile_pool(name=f"s0{layer}", bufs=2) as sel0p, \
                         tc.tile_pool(name=f"mt{layer}", bufs=6) as mtp, \
                         tc.tile_pool(name=f"ho{layer}", bufs=4) as hop, \
                         tc.tile_pool(name=f"pa{layer}", bufs=4, space="PSUM") as pap, \
                         tc.tile_pool(name=f"pb{layer}", bufs=2, space="PSUM") as pbp:
                        for pr in range(NP_):
                            pch0 = pr * NCHUNK
                            cb_pair = int(colbase_pch[pch0])
                            ncol_pair = int(colbase_pch[pch0 + NCHUNK]) - cb_pair
                            tb_pair = int(tile_base[pr, 0])
                            nt_pair = int(nt[pr].sum())

                            # pair-level loads: idx + slot + ce slabs
                            it = idxp.tile([P, NTMAXG * NCHUNK * 8], i16, tag="idx")
                            sl = scp.tile([P, CMAXC * NCHUNK], bf16, tag="slot")
                            cv = scp.tile([P, CMAXC * NCHUNK], bf16, tag="ce")
                            nc.sync.dma_start(
                                sl[:, :ncol_pair],
                                slot_d[:, cb_pair:cb_pair + ncol_pair])
                            nc.sync.dma_start(
                                cv[:, :ncol_pair],
                                ce_d[:, cb_pair:cb_pair + ncol_pair])

                            # psa accumulators: one PSUM bank per window holds
                            # all 4 relations side by side ([128, 512] f32).
                            # The start=True matmul of a group clears the
                            # has_written bits of the WHOLE bank, so the 4
                            # relation groups per bank must run sequentially
                            # (group-major matmul order, all chunks live).
                            psa_bank = [
                                pap.tile([P, R * P], f32, tag="pa",
                                         name=f"pa{layer}_{pr}_{wl}")
                                for wl in range(2)]
                            psa = {(wl, r): psa_bank[wl][:, r * P:(r + 1) * P]
                                   for wl in range(2) for r in range(R)}

                            xs = {}
                            sels = {}
                            for ch in range(NCHUNK):
                                pch = pch0 + ch
                                ntc = int(nt[pr, ch])
                                q = pch % 4
                                off8 = (int(tile_base[pr, ch]) - tb_pair) * 8
                                nidx = ntc * P
                                # idx band for queue q only (or full replica)
                                tb8 = int(tile_base[pr, ch]) * 8
                                if _KBAND:
                                    nc.sync.dma_start(
                                        it[32 * q:32 * (q + 1),
                                           off8:off8 + ntc * 8],
                                        gidx_d[:, tb8:tb8 + ntc * 8])
                                else:
                                    nc.sync.dma_start(
                                        it[:, off8:off8 + ntc * 8],
                                        gidx_d[:, tb8:tb8 + ntc * 8])

                                x = xp_g.tile([P, NTMAXG, D], bf16, tag="x")
                                nc.gpsimd.dma_gather(
                                    x[:, :ntc, :],
                                    table[ch * CHUNK:(ch + 1) * CHUNK],
                                    it[:, off8:off8 + ntc * 8],
                                    nidx, nidx_regs[nidx], D,
                                    single_packet=False, queue_num=q)
                                xs[ch] = x

                                # on-chip sel generation (DVE, 2 passes)
                                cb_ch = int(colbase_pch[pch]) - cb_pair
                                ncol = int(colbase_pch[pch + 1]) - int(colbase_pch[pch])
                                sel0 = sel0p.tile([P, CMAXC, P], bf16, tag="sel0")
                                sel = selp.tile([P, CMAXC, P], bf16, tag="sel")
                                nc.vector.tensor_tensor(
                                    out=sel0[:, :ncol, :],
                                    in0=iota_sb[:, :ncol, :],
                                    in1=sl[:, cb_ch:cb_ch + ncol, None]
                                        .to_broadcast([P, ncol, P]),
                                    op=mybir.AluOpType.is_equal)
                                nc.vector.tensor_tensor(
                                    out=sel[:, :ncol, :],
                                    in0=sel0[:, :ncol, :],
                                    in1=cv[:, cb_ch:cb_ch + ncol, None]
                                        .to_broadcast([P, ncol, P]),
                                    op=mybir.AluOpType.mult)
                                sels[ch] = sel

                            # aggregation matmuls, (window, relation)-major so
                            # each bank's 4 groups are strictly sequential
                            groups = {}
                            for ch in range(NCHUNK):
                                for (wlr, t, c) in cols_per_pch[pch0 + ch]:
                                    groups.setdefault(wlr, []).append(
                                        (ch, t, c - int(colbase_pch[pch0 + ch])))
                            for wlr in sorted(groups):
                                wl, r = wlr >> 2, wlr & 3
                                plist = groups[wlr]
                                for i, (ch, t, j) in enumerate(plist):
                                    nc.tensor.matmul(
                                        psa[(wl, r)],
                                        lhsT=xs[ch][:, t, :],
                                        rhs=sels[ch][:, j, :],
                                        start=(i == 0),
                                        stop=(i == len(plist) - 1))

                            # weight matmuls + relu + store, per window
                            for wl in range(2):
                                w_glob = pr * 2 + wl
                                psb = pbp.tile([P, P], f32, tag="pb")
                                for r in range(R):
                                    mt = mtp.tile([P, P], bf16, tag="mt")
                                    nc.scalar.copy(mt[:], psa[(wl, r)])
                                    nc.tensor.matmul(
                                        psb[:], lhsT=mt[:],
                                        rhs=w_sb[layer * R + r][:],
                                        start=(r == 0), stop=(r == R - 1))
                                rows = P if w_glob < NW - 1 else LAST_W_ROWS
                                ho = hop.tile([P, D], bf16, tag="ho")
                                nc.scalar.activation(
                                    ho[:], psb[:],
                                    mybir.ActivationFunctionType.Relu)
                                nc.scalar.dma_start(
                                    out_slice[w_glob * P:w_glob * P + rows],
                                    ho[:rows, :])

                run_layer(0, feat, h_slice[0])
                tc.strict_bb_all_engine_barrier()
                if _KLAYERS >= 2:
                    with tc.tile_critical():
                        cc_count[0] += 1
                        nc.gpsimd.collective_compute(
                            "AllGather", mybir.AluOpType.bypass,
                            ins=[h_slice[0][:]], outs=[h_full[1][:]],
                            replica_groups=[list(range(NCORES))],
                        ).then_inc(cc_sem, 1)
                        nc.gpsimd.wait_ge(cc_sem, cc_count[0])
                    tc.strict_bb_all_engine_barrier()

                    run_layer(1, h_full[1], h_slice[1])
                    tc.strict_bb_all_engine_barrier()
                if _KLAYERS >= 3:
                    with tc.tile_critical():
                        cc_count[0] += 1
                        nc.gpsimd.collective_compute(
                            "AllGather", mybir.AluOpType.bypass,
                            ins=[h_slice[1][:]], outs=[h_full[2][:]],
                            replica_groups=[list(range(NCORES))],
                        ).then_inc(cc_sem, 1)
                        nc.gpsimd.wait_ge(cc_sem, cc_count[0])
                    tc.strict_bb_all_engine_barrier()

                    run_layer(2, h_full[2], h_slice[2])
                    tc.strict_bb_all_engine_barrier()

            # ---- pooling + classifier ----
            with tc.tile_pool(name="poolp", bufs=6) as pp, \
                 tc.tile_pool(name="pps", bufs=2, space="PSUM") as pps:
                psc = pps.tile([P, G], f32, tag="psc")
                h_pool = h_slice[_KLAYERS - 1]
                for w in range(NW):
                    rows = P if w < NW - 1 else LAST_W_ROWS
                    ht = pp.tile([P, D], bf16, tag="ht")
                    nc.sync.dma_start(ht[:rows, :],
                                      h_pool[w * P:w * P + rows])
                    sg = pp.tile([P, G], bf16, tag="sg")
                    nc.sync.dma_start(sg[:], psel_d[:, w * G:(w + 1) * G])
                    nc.tensor.matmul(psc[:], lhsT=ht[:rows, :],
                                     rhs=sg[:rows, :],
                                     start=(w == 0), stop=(w == NW - 1))
                pool_sb = pp.tile([P, G], f32, tag="poolsb")
                nc.scalar.copy(pool_sb[:], psc[:])
                nc.sync.dma_start(poolin[:], pool_sb[:])
                tc.strict_bb_all_engine_barrier()
                with tc.tile_critical():
                    cc_count[0] += 1
                    nc.gpsimd.collective_compute(
                        "AllReduce", mybir.AluOpType.add,
                        ins=[poolin[:]], outs=[poolout[:]],
                        replica_groups=[list(range(NCORES))],
                    ).then_inc(cc_sem, 1)
                    nc.gpsimd.wait_ge(cc_sem, cc_count[0])
                tc.strict_bb_all_engine_barrier()
                pout = pp.tile([P, G], f32, tag="pout")
                nc.sync.dma_start(pout[:], poolout[:])
                pcls = pps.tile([NCLS, G], f32, tag="pcls")
                nc.tensor.matmul(pcls[:], lhsT=wc_sb[:], rhs=pout[:],
                                 start=True, stop=True)
                osb = pp.tile([NCLS, G], f32, tag="osb")
                nc.scalar.activation(
                    osb[:], pcls[:],
                    mybir.ActivationFunctionType.Identity, bias=bc_sb[:])
                nc.sync.dma_start(out_d[:], osb[:])

    nc.compile()
    return nc


def _get_compiled(inputs):
    key = "k"
    if key in _CACHE:
        return _CACHE[key]
    sched, arrays = _prep(inputs["edges"], inputs["graph_ids"])
    nc = _build(sched)
    _CACHE[key] = (nc, sched, arrays)
    return _CACHE[key]


def _in_maps(inputs, arrays):
    import ml_dtypes
    bf = ml_dtypes.bfloat16
    feat = np.ascontiguousarray(
        np.asarray(inputs["features"], dtype=np.float32).astype(bf))
    W0 = np.asarray(inputs["W0"], dtype=np.float32)
    Wl = np.asarray(inputs["Wl"], dtype=np.float32)
    wstack = np.concatenate([W0.reshape(R, D, D),
                             Wl.reshape((L - 1) * R, D, D)], axis=0).astype(bf)
    b0 = np.asarray(inputs["b0"])
    bl = np.asarray(inputs["bl"])
    assert np.all(b0 == 0) and np.all(bl == 0), \
        "nonzero per-relation biases not folded in this kernel"
    wc = np.asarray(inputs["Wc"], dtype=np.float32)
    bcb = np.asarray(inputs["bc"], dtype=np.float32).reshape(NCLS, 1)
    maps = []
    for c in range(NCORES):
        maps.append({
            "feat": feat, "wstack": wstack, "wc": wc, "bcb": bcb,
            "gidx": arrays["gidx"][c],
            "slotslab": arrays["slotslab"][c],
            "ceslab": arrays["ceslab"][c],
            "iota": arrays["iota"],
            "psel": arrays["psel"][c].reshape(P, NW * G),
        })
    return maps


def kernel(**inputs) -> np.ndarray:
    nc, sched, arrays = _get_compiled(inputs)
    maps = _in_maps(inputs, arrays)
    res = run_bass_kernel_spmd(nc, maps, list(range(NCORES)), trace=False)
    return np.ascontiguousarray(res.results[0]["out"].T)


def kernel_traced(**inputs):
    """Like kernel() but returns (output, exec_time_ns). Used by test.py."""
    import types
    import concourse.bass_utils as bum
    if "antenv.axon_hooks" not in sys.modules:
        mod = types.ModuleType("antenv.axon_hooks")
        mod._hook = None
        mod.set_axon_ntff_profile_hook = lambda h: setattr(mod, "_hook", h)
        mod.get_axon_ntff_profile_hook = lambda: mod._hook
        sys.modules["antenv.axon_hooks"] = mod
        import antenv
        antenv.axon_hooks = mod
        from trn_agent_boot.trn_boot import _ntff_profile_via_ctypes
        mod._hook = _ntff_profile_via_ctypes('/opt/axon/libaxon_pjrt.so')
    bum.upload_artifacts = lambda tmpdir: "local://skipped"
    nc, sched, arrays = _get_compiled(inputs)
    maps = _in_maps(inputs, arrays)
    res = run_bass_kernel_spmd(nc, maps, list(range(NCORES)), trace=True)
    return np.ascontiguousarray(res.results[0]["out"].T), res.exec_time_ns


# revision 22
# speedup vs baseline: 1.0306x; 1.0306x over previous
"""Trainium2 Bass kernel for nn_Classifier_36618891166176 (R-GCN message passing).

Strategy (8 NeuronCores, SPMD single program):
  - Partition dst nodes across cores (12500 each). Each core processes the
    edges whose dst lies in its range, for all 4 relations and all 3 layers.
  - Per layer, per (window-pair, src-chunk): ONE dma_gather call fetches the
    h[src] rows for all edges of both windows and all 4 relations (edges
    sorted by (window, relation), trailing -1 indices are stripped by the Q7
    ucode so each core only pays descriptors for its true edge count).
  - The one-hot aggregation matrices (sel) are generated ON-CHIP on the idle
    vector engine: sel = is_equal(IOTA, slot) * ce, two batched tensor_tensor
    passes with stride-0 broadcast of per-column slot/ce vectors. Rows not
    belonging to a (window, relation) piece have slot=-1 -> sel row 0, which
    also masks stale gather rows. This removes the 72MB/layer sel stream of
    the previous version entirely.
  - Aggregation per piece (tile, window, relation): matmul(psa[w][r],
    lhsT=x_tile, rhs=sel_col) accumulated in PSUM; then per window the
    per-relation weight matmuls, relu, store; AllGather into shared h.
  - AvgPool via host-precomputed pool-sel matmul + AllReduce, as before.

dma_gather indices are int16 (max 32768 rows), so h is addressed in 4 chunks
of 25000 rows. The index tile only needs the 32-partition band of the queue
that serves the call (Q7 cores 2q, 2q+1), so the host index slab holds 2
replicas instead of 8.
"""
import os
import sys
import numpy as np

import concourse.bacc as bacc
import concourse.bass as bass
import concourse.mybir as mybir
import concourse.tile as tile
from concourse.bass_utils import run_bass_kernel_spmd
from concourse import library_config

# Problem constants (hardcoded per harness contract).
N = 100000
E = 1000000
R = 4
G = 256
D = 128          # feature dim (DIN == DH == 128)
NCLS = 10
L = 3
NCORES = 8
NSLICE = N // NCORES          # 12500 dst nodes per core
NW = (NSLICE + 127) // 128    # 98 dst windows per core
LAST_W_ROWS = NSLICE - (NW - 1) * 128  # 84
NP_ = NW // 2                 # 49 window pairs
CHUNK = 25000
NCHUNK = (N + CHUNK - 1) // CHUNK      # 4
P = 128

_CACHE = {}

# bisection knobs (default = full-featured kernel)
_KLAYERS = int(os.environ.get("KLAYERS", "3"))
_KBAND = int(os.environ.get("KBAND", "1"))    # 1: 32-part band idx, 0: 8-replica
# NOTE: trailing -1 stripping (KSTRIP=1) hangs the SWDGE ring: the decode
# side reserves ring space from the num_idxs register while the Q7 kernel
# writes descriptors per the value-stripped count; the bookkeeping drift
# wedges the queue after enough calls. Zero-padding is mandatory.
_KSTRIP = int(os.environ.get("KSTRIP", "0"))  # 1: -1 pad (stripped), 0: idx-0 pad
_KSCRATCH = int(os.environ.get("KSCRATCH", "65536"))


def _prep(edges, graph_ids):
    """Host-side edge preprocessing. Layer-invariant. Returns per-core arrays
    plus the uniform schedule."""
    import ml_dtypes
    bf = ml_dtypes.bfloat16
    edges = np.asarray(edges)
    graph_ids = np.asarray(graph_ids)

    # Flatten all relations' edges with per-edge normalization coefficient.
    srcs = []
    dsts = []
    ces = []
    rels = []
    for r in range(R):
        src = edges[r, 0].astype(np.int64)
        dst = edges[r, 1].astype(np.int64)
        deg_out = np.maximum(np.bincount(src, minlength=N), 1.0)
        deg_in = np.maximum(np.bincount(dst, minlength=N), 1.0)
        ce = (1.0 / np.sqrt(deg_out[src]) / np.sqrt(deg_in[dst])).astype(np.float32)
        srcs.append(src)
        dsts.append(dst)
        ces.append(ce)
        rels.append(np.full(E, r, dtype=np.int64))
    src = np.concatenate(srcs)
    dst = np.concatenate(dsts)
    ce = np.concatenate(ces)
    rel = np.concatenate(rels)

    core = dst // NSLICE
    local = dst - core * NSLICE
    w = local >> 7
    slot = local & 127
    pair = w >> 1
    wl = w & 1
    ch = src // CHUNK
    srcl = (src % CHUNK).astype(np.int64)

    # Sort edges by (core, pair, ch, wl, rel).
    key = ((((core * NP_ + pair) * NCHUNK + ch) * 2 + wl) * R + rel)
    order = np.argsort(key, kind="stable")
    key = key[order]
    srcl = srcl[order]
    slot = slot[order]
    ce = ce[order]

    NSEG = NCORES * NP_ * NCHUNK          # (core, pair, ch) segments
    seg_key = key // (2 * R)
    seg_bounds = np.searchsorted(seg_key, np.arange(NSEG + 1))
    seg_counts = np.diff(seg_bounds)      # [NSEG]
    seg_start = seg_bounds[:-1]
    # rank of each edge within its (core, pair, ch) segment
    k_local = np.arange(len(key)) - np.repeat(seg_start, seg_counts)

    # nt per (pair, ch) = ceil(max over cores / 128)
    cnt = seg_counts.reshape(NCORES, NP_, NCHUNK)
    nt = np.maximum(1, -(-cnt.max(axis=0) // 128))   # [NP_, NCHUNK]
    flat = nt.reshape(-1)
    tb = np.concatenate([[0], np.cumsum(flat)[:-1]])
    tile_base = tb.reshape(NP_, NCHUNK)
    TT = int(flat.sum())
    EPAD = TT * P

    # Subcell bounds per (core, pair, ch, wl, r) for piece enumeration.
    NSUB = NSEG * 2 * R
    sub_bounds = np.searchsorted(key, np.arange(NSUB + 1))
    sub_a = sub_bounds[:-1]
    sub_b = sub_bounds[1:]
    # local (within segment) bounds
    seg_of_sub = np.arange(NSUB) // (2 * R)
    la = sub_a - seg_start[seg_of_sub]
    lb = sub_b - seg_start[seg_of_sub]

    # Pieces per (pair, ch): union over cores of (wl, r, t) tile spans.
    # coltab[(pair, ch, wl, r), t] -> global column id or -1
    pieces = {}   # (pair, ch) -> list of (wl, r, t)
    for pch in range(NP_ * NCHUNK):
        pieces[pch] = set()
    sub_ids = np.arange(NSUB)
    core_of_sub = sub_ids // (NP_ * NCHUNK * 2 * R)
    rem = sub_ids % (NP_ * NCHUNK * 2 * R)
    pch_of_sub = rem // (2 * R)
    wlr_of_sub = rem % (2 * R)
    nz = lb > la
    for i in np.nonzero(nz)[0]:
        t0 = la[i] // P
        t1 = (lb[i] - 1) // P
        s = pieces[pch_of_sub[i]]
        wlr = wlr_of_sub[i]
        for t in range(t0, t1 + 1):
            s.add((wlr, t))

    # Assign global column ids, ordered by (pair, ch, wl, r, t).
    col_of = {}       # (pch, wlr, t) -> global col
    cols_per_pch = []  # pch -> ordered piece list [(wlr, t, col)]
    colbase_pch = np.zeros(NP_ * NCHUNK + 1, dtype=np.int64)
    cid = 0
    for pch in range(NP_ * NCHUNK):
        colbase_pch[pch] = cid
        lst = sorted(pieces[pch])
        out = []
        for (wlr, t) in lst:
            col_of[(pch, wlr, t)] = cid
            out.append((wlr, t, cid))
            cid += 1
        cols_per_pch.append(out)
    colbase_pch[NP_ * NCHUNK] = cid
    TOTCOL = cid

    # Per-edge column id for the slot/ce scatter.
    # edge -> (pch, wlr, t_e) where t_e = k_local // 128
    pch_e = seg_key % (NP_ * NCHUNK)      # (pair, ch), core stripped
    wlr_e = key % (2 * R)
    t_e = k_local // P
    # build lookup array: max tiles per pch
    NTMAXG = int(nt.max())
    lut = np.full((NP_ * NCHUNK * 2 * R, NTMAXG + 2), -1, dtype=np.int64)
    for (pch, wlr, t), c in col_of.items():
        lut[pch * 2 * R + wlr, t] = c
    colidx_e = lut[pch_e * 2 * R + wlr_e, t_e]
    assert colidx_e.min() >= 0

    # Per-core arrays.
    core_e = seg_key // (NP_ * NCHUNK)
    part_e = k_local % P

    slotslab = np.full((NCORES, P, TOTCOL), -1.0, dtype=bf)
    slotslab[core_e, part_e, colidx_e] = slot.astype(bf)
    # per-tile ce vectors (ce folded into x tiles on the scalar engine)
    tg_e = tile_base[(pch_e // NCHUNK), (pch_e % NCHUNK)] + t_e
    cex = np.zeros((NCORES, P, TT), dtype=np.float32)
    cex[core_e, part_e, tg_e] = ce

    # Gather index slab: [NCORES, 32, EPAD/16] int16, init -1 (trailing strip).
    # Index k of call (pair, ch) sits at (16*rep + k%16, tile_base*8 + k//16),
    # rep in {0, 1} (only the call's queue band is ever read; the DMA writes
    # the band for queue q = (pair*NCHUNK+ch) % 4).
    nrep = 2 if _KBAND else 8
    fillv = -1 if _KSTRIP else 0
    gidx = np.full((NCORES, 16 * nrep, EPAD // 16), fillv, dtype=np.int16)
    tb_e = tile_base[(pch_e // NCHUNK), (pch_e % NCHUNK)]
    gcol = tb_e * 8 + k_local // 16
    grow = (k_local % 16).astype(np.int64)
    s16 = srcl.astype(np.int16)
    for rep in range(nrep):
        gidx[core_e, grow + 16 * rep, gcol] = s16

    # Pool sel slabs: [128, NW, 256] bf16 per core (1/count at graph id).
    counts = np.maximum(np.bincount(graph_ids.astype(np.int64), minlength=G),
                        1.0).astype(np.float32)
    psel = np.zeros((NCORES, P, NW, G), dtype=bf)
    for c in range(NCORES):
        g = graph_ids[c * NSLICE:(c + 1) * NSLICE].astype(np.int64)
        nodes = np.arange(NSLICE)
        wv = nodes >> 7
        pv = nodes & 127
        psel[c, pv, wv, g] = (1.0 / counts[g]).astype(bf)

    # max columns within one (pair, ch) chunk-slab
    CMAXC = int(np.diff(colbase_pch).max())

    sched = {
        "nt": nt, "tile_base": tile_base, "TT": TT, "EPAD": EPAD,
        "cols_per_pch": cols_per_pch, "colbase_pch": colbase_pch,
        "TOTCOL": TOTCOL, "CMAXC": CMAXC, "NTMAXG": NTMAXG,
    }
    # IOTA constant [128, CMAXC*128] bf16: per partition, 0..127 repeated.
    iota = np.tile(np.arange(P, dtype=np.float32), CMAXC)
    iota = np.broadcast_to(iota, (P, CMAXC * P)).astype(bf)
    arrays = {
        "gidx": gidx, "slotslab": slotslab, "cex": cex, "psel": psel,
        "iota": np.ascontiguousarray(iota),
    }
    return sched, arrays


def _build(sched):
    """Build the SPMD bass program for the uniform schedule."""
    nt = sched["nt"]
    tile_base = sched["tile_base"]
    EPAD = sched["EPAD"]
    cols_per_pch = sched["cols_per_pch"]
    colbase_pch = sched["colbase_pch"]
    TOTCOL = sched["TOTCOL"]
    CMAXC = sched["CMAXC"]
    NTMAXG = sched["NTMAXG"]

    f32 = mybir.dt.float32
    bf16 = mybir.dt.bfloat16
    i16 = mybir.dt.int16

    nc = bacc.Bacc("TRN2", target_bir_lowering=False, debug=False,
                   num_swdge_queues=4, dynamic_dma_scratch_size=_KSCRATCH)

    feat = nc.dram_tensor("feat", [N, D], bf16, kind="ExternalInput")
    wstack = nc.dram_tensor("wstack", [L * R, D, D], bf16, kind="ExternalInput")
    wc = nc.dram_tensor("wc", [D, NCLS], f32, kind="ExternalInput")
    bcb = nc.dram_tensor("bcb", [NCLS, 1], f32, kind="ExternalInput")
    IDXROWS = 32 if _KBAND else P
    gidx_d = nc.dram_tensor("gidx", [IDXROWS, EPAD // 16], i16,
                            kind="ExternalInput")
    slot_d = nc.dram_tensor("slotslab", [P, TOTCOL], bf16, kind="ExternalInput")
    cex_d = nc.dram_tensor("cex", [P, sched["TT"]], f32, kind="ExternalInput")
    iota_d = nc.dram_tensor("iota", [P, CMAXC * P], bf16, kind="ExternalInput")
    psel_d = nc.dram_tensor("psel", [P, NW * G], bf16, kind="ExternalInput")

    h_full = [None,
              nc.dram_tensor("h1f", [N, D], bf16, addr_space="Shared"),
              nc.dram_tensor("h2f", [N, D], bf16, addr_space="Shared")]
    h_slice = [nc.dram_tensor(f"hs{l}", [NSLICE, D], bf16) for l in range(L)]
    poolin = nc.dram_tensor("poolin", [P, G], f32)
    poolout = nc.dram_tensor("poolout", [P, G], f32, addr_space="Shared")
    out_d = nc.dram_tensor("out", [NCLS, G], f32, kind="ExternalOutput")

    cc_sem = nc.alloc_semaphore("ccsem")
    cc_count = [0]

    with tile.TileContext(nc) as tc:
        nc.gpsimd.load_library(library_config.mlp)

        with tc.tile_pool(name="const", bufs=1) as cpool:
            w_sb = []
            for i in range(L * R):
                t = cpool.tile([D, D], bf16, tag=f"w{i}")
                nc.sync.dma_start(t[:], wstack[i])
                w_sb.append(t)
            wc_sb = cpool.tile([D, NCLS], f32, tag="wc")
            nc.sync.dma_start(wc_sb[:], wc[:])
            bc_sb = cpool.tile([NCLS, 1], f32, tag="bc")
            nc.sync.dma_start(bc_sb[:], bcb[:])
            iota_sb = cpool.tile([P, CMAXC, P], bf16, tag="iota")
            nc.sync.dma_start(iota_sb[:], iota_d[:])

            nidx_regs = {}
            for p_ in range(NP_):
                for c_ in range(NCHUNK):
                    v = int(nt[p_, c_]) * P
                    if v not in nidx_regs:
                        nidx_regs[v] = nc.gpsimd.to_reg(v)

            XBUFS = 8
            with tc.tile_pool(name="xg", bufs=XBUFS) as xp_g:
                # memset the x pool once: stale (unwritten) rows are fed to
                # the PE multiplied by sel=0; they must never be NaN bits.
                for _ in range(XBUFS):
                    t = xp_g.tile([P, NTMAXG, D], bf16, tag="x")
                    nc.vector.memset(t[:], 0.0)

                def run_layer(layer, table, out_slice):
                    with tc.tile_pool(name=f"idx{layer}", bufs=4) as idxp, \
                         tc.tile_pool(name=f"sc{layer}", bufs=4) as scp, \
                         tc.tile_pool(name=f"sel{layer}", bufs=6) as selp, \
                         tc.tile_pool(name=f"mt{layer}", bufs=6) as mtp, \
                         tc.tile_pool(name=f"ho{layer}", bufs=4) as hop, \
                         tc.tile_pool(name=f"pa{layer}", bufs=4, space="PSUM") as pap, \
                         tc.tile_pool(name=f"pb{layer}", bufs=2, space="PSUM") as pbp:
                        for pr in range(NP_):
                            pch0 = pr * NCHUNK
                            cb_pair = int(colbase_pch[pch0])
                            ncol_pair = int(colbase_pch[pch0 + NCHUNK]) - cb_pair
                            tb_pair = int(tile_base[pr, 0])
                            nt_pair = int(nt[pr].sum())

                            # pair-level loads: idx + slot + ce slabs
                            it = idxp.tile([P, NTMAXG * NCHUNK * 8], i16, tag="idx")
                            sl = scp.tile([P, CMAXC * NCHUNK], bf16, tag="slot")
                            cxt = scp.tile([P, NTMAXG * NCHUNK], f32, tag="cex")
                            nc.sync.dma_start(
                                sl[:, :ncol_pair],
                                slot_d[:, cb_pair:cb_pair + ncol_pair])
                            nc.sync.dma_start(
                                cxt[:, :nt_pair],
                                cex_d[:, tb_pair:tb_pair + nt_pair])

                            # psa accumulators: one PSUM bank per window holds
                            # all 4 relations side by side ([128, 512] f32).
                            # The start=True matmul of a group clears the
                            # has_written bits of the WHOLE bank, so the 4
                            # relation groups per bank must run sequentially
                            # (group-major matmul order, all chunks live).
                            psa_bank = [
                                pap.tile([P, R * P], f32, tag="pa",
                                         name=f"pa{layer}_{pr}_{wl}")
                                for wl in range(2)]
                            psa = {(wl, r): psa_bank[wl][:, r * P:(r + 1) * P]
                                   for wl in range(2) for r in range(R)}

                            xs = {}
                            sels = {}
                            for ch in range(NCHUNK):
                                pch = pch0 + ch
                                ntc = int(nt[pr, ch])
                                q = pch % 4
                                off8 = (int(tile_base[pr, ch]) - tb_pair) * 8
                                nidx = ntc * P
                                # idx band for queue q only (or full replica)
                                tb8 = int(tile_base[pr, ch]) * 8
                                if _KBAND:
                                    nc.sync.dma_start(
                                        it[32 * q:32 * (q + 1),
                                           off8:off8 + ntc * 8],
                                        gidx_d[:, tb8:tb8 + ntc * 8])
                                else:
                                    nc.sync.dma_start(
                                        it[:, off8:off8 + ntc * 8],
                                        gidx_d[:, tb8:tb8 + ntc * 8])

                                x = xp_g.tile([P, NTMAXG, D], bf16, tag="x")
                                nc.gpsimd.dma_gather(
                                    x[:, :ntc, :],
                                    table[ch * CHUNK:(ch + 1) * CHUNK],
                                    it[:, off8:off8 + ntc * 8],
                                    nidx, nidx_regs[nidx], D,
                                    single_packet=False, queue_num=q)
                                # fold ce into x rows (ACT, per tile, own port)
                                tb_loc = int(tile_base[pr, ch]) - tb_pair
                                for tloc in range(ntc):
                                    nc.scalar.activation(
                                        x[:, tloc, :], x[:, tloc, :],
                                        mybir.ActivationFunctionType.Identity,
                                        scale=cxt[:, tb_loc + tloc:tb_loc + tloc + 1])
                                xs[ch] = x

                                # on-chip 0/1 sel generation (DVE, 1 pass)
                                cb_ch = int(colbase_pch[pch]) - cb_pair
                                ncol = int(colbase_pch[pch + 1]) - int(colbase_pch[pch])
                                sel = selp.tile([P, CMAXC, P], bf16, tag="sel")
                                nc.vector.tensor_tensor(
                                    out=sel[:, :ncol, :],
                                    in0=iota_sb[:, :ncol, :],
                                    in1=sl[:, cb_ch:cb_ch + ncol, None]
                                        .to_broadcast([P, ncol, P]),
                                    op=mybir.AluOpType.is_equal)
                                sels[ch] = sel

                            # aggregation matmuls, (window, relation)-major so
                            # each bank's 4 groups are strictly sequential
                            groups = {}
                            for ch in range(NCHUNK):
                                for (wlr, t, c) in cols_per_pch[pch0 + ch]:
                                    groups.setdefault(wlr, []).append(
                                        (ch, t, c - int(colbase_pch[pch0 + ch])))
                            for wlr in sorted(groups):
                                wl, r = wlr >> 2, wlr & 3
                                plist = groups[wlr]
                                for i, (ch, t, j) in enumerate(plist):
                                    nc.tensor.matmul(
                                        psa[(wl, r)],
                                        lhsT=xs[ch][:, t, :],
                                        rhs=sels[ch][:, j, :],
                                        start=(i == 0),
                                        stop=(i == len(plist) - 1))

                            # weight matmuls + relu + store, per window
                            for wl in range(2):
                                w_glob = pr * 2 + wl
                                psb = pbp.tile([P, P], f32, tag="pb")
                                for r in range(R):
                                    mt = mtp.tile([P, P], bf16, tag="mt")
                                    nc.scalar.copy(mt[:], psa[(wl, r)])
                                    nc.tensor.matmul(
                                        psb[:], lhsT=mt[:],
                                        rhs=w_sb[layer * R + r][:],
                                        start=(r == 0), stop=(r == R - 1))
                                rows = P if w_glob < NW - 1 else LAST_W_ROWS
                                ho = hop.tile([P, D], bf16, tag="ho")
                                nc.scalar.activation(
                                    ho[:], psb[:],
                                    mybir.ActivationFunctionType.Relu)
                                nc.scalar.dma_start(
                                    out_slice[w_glob * P:w_glob * P + rows],
                                    ho[:rows, :])

                run_layer(0, feat, h_slice[0])
                tc.strict_bb_all_engine_barrier()
                if _KLAYERS >= 2:
                    with tc.tile_critical():
                        cc_count[0] += 1
                        nc.gpsimd.collective_compute(
                            "AllGather", mybir.AluOpType.bypass,
                            ins=[h_slice[0][:]], outs=[h_full[1][:]],
                            replica_groups=[list(range(NCORES))],
                        ).then_inc(cc_sem, 1)
                        nc.gpsimd.wait_ge(cc_sem, cc_count[0])
                    tc.strict_bb_all_engine_barrier()

                    run_layer(1, h_full[1], h_slice[1])
                    tc.strict_bb_all_engine_barrier()
                if _KLAYERS >= 3:
                    with tc.tile_critical():
                        cc_count[0] += 1
                        nc.gpsimd.collective_compute(
                            "AllGather", mybir.AluOpType.bypass,
                            ins=[h_slice[1][:]], outs=[h_full[2][:]],
                            replica_groups=[list(range(NCORES))],
                        ).then_inc(cc_sem, 1)
                        nc.gpsimd.wait_ge(cc_sem, cc_count[0])
                    tc.strict_bb_all_engine_barrier()

                    run_layer(2, h_full[2], h_slice[2])
                    tc.strict_bb_all_engine_barrier()

            # ---- pooling + classifier ----
            with tc.tile_pool(name="poolp", bufs=6) as pp, \
                 tc.tile_pool(name="pps", bufs=2, space="PSUM") as pps:
                psc = pps.tile([P, G], f32, tag="psc")
                h_pool = h_slice[_KLAYERS - 1]
                for w in range(NW):
                    rows = P if w < NW - 1 else LAST_W_ROWS
                    ht = pp.tile([P, D], bf16, tag="ht")
                    nc.sync.dma_start(ht[:rows, :],
                                      h_pool[w * P:w * P + rows])
                    sg = pp.tile([P, G], bf16, tag="sg")
                    nc.sync.dma_start(sg[:], psel_d[:, w * G:(w + 1) * G])
                    nc.tensor.matmul(psc[:], lhsT=ht[:rows, :],
                                     rhs=sg[:rows, :],
                                     start=(w == 0), stop=(w == NW - 1))
                pool_sb = pp.tile([P, G], f32, tag="poolsb")
                nc.scalar.copy(pool_sb[:], psc[:])
                nc.sync.dma_start(poolin[:], pool_sb[:])
                tc.strict_bb_all_engine_barrier()
                with tc.tile_critical():
                    cc_count[0] += 1
                    nc.gpsimd.collective_compute(
                        "AllReduce", mybir.AluOpType.add,
                        ins=[poolin[:]], outs=[poolout[:]],
                        replica_groups=[list(range(NCORES))],
                    ).then_inc(cc_sem, 1)
                    nc.gpsimd.wait_ge(cc_sem, cc_count[0])
                tc.strict_bb_all_engine_barrier()
                pout = pp.tile([P, G], f32, tag="pout")
                nc.sync.dma_start(pout[:], poolout[:])
                pcls = pps.tile([NCLS, G], f32, tag="pcls")
                nc.tensor.matmul(pcls[:], lhsT=wc_sb[:], rhs=pout[:],
                                 start=True, stop=True)
                osb = pp.tile([NCLS, G], f32, tag="osb")
                nc.scalar.activation(
                    osb[:], pcls[:],
                    mybir.ActivationFunctionType.Identity, bias=bc_sb[:])
                nc.sync.dma_start(out_d[:], osb[:])

    nc.compile()
    return nc


def _get_compiled(inputs):
    key = "k"
    if key in _CACHE:
        return _CACHE[key]
    sched, arrays = _prep(inputs["edges"], inputs["graph_ids"])
    nc = _build(sched)
    _CACHE[key] = (nc, sched, arrays)
    return _CACHE[key]


def _in_maps(inputs, arrays):
    import ml_dtypes
    bf = ml_dtypes.bfloat16
    feat = np.ascontiguousarray(
        np.asarray(inputs["features"], dtype=np.float32).astype(bf))
    W0 = np.asarray(inputs["W0"], dtype=np.float32)
    Wl = np.asarray(inputs["Wl"], dtype=np.float32)
    wstack = np.concatenate([W0.reshape(R, D, D),
                             Wl.reshape((L - 1) * R, D, D)], axis=0).astype(bf)
    b0 = np.asarray(inputs["b0"])
    bl = np.asarray(inputs["bl"])
    assert np.all(b0 == 0) and np.all(bl == 0), \
        "nonzero per-relation biases not folded in this kernel"
    wc = np.asarray(inputs["Wc"], dtype=np.float32)
    bcb = np.asarray(inputs["bc"], dtype=np.float32).reshape(NCLS, 1)
    maps = []
    for c in range(NCORES):
        maps.append({
            "feat": feat, "wstack": wstack, "wc": wc, "bcb": bcb,
            "gidx": arrays["gidx"][c],
            "slotslab": arrays["slotslab"][c],
            "cex": arrays["cex"][c],
            "iota": arrays["iota"],
            "psel": arrays["psel"][c].reshape(P, NW * G),
        })
    return maps


def kernel(**inputs) -> np.ndarray:
    nc, sched, arrays = _get_compiled(inputs)
    maps = _in_maps(inputs, arrays)
    res = run_bass_kernel_spmd(nc, maps, list(range(NCORES)), trace=False)
    return np.ascontiguousarray(res.results[0]["out"].T)


def kernel_traced(**inputs):
    """Like kernel() but returns (output, exec_time_ns). Used by test.py."""
    import types
    import concourse.bass_utils as bum
    if "antenv.axon_hooks" not in sys.modules:
        mod = types.ModuleType("antenv.axon_hooks")
        mod._hook = None
        mod.set_axon_ntff_profile_hook = lambda h: setattr(mod, "_hook", h)
        mod.get_axon_ntff_profile_hook = lambda: mod._hook
        sys.modules["antenv.axon_hooks"] = mod
        import antenv
        antenv.axon_hooks = mod
        from trn_agent_boot.trn_boot import _ntff_profile_via_ctypes
        mod._hook = _ntff_profile_via_ctypes('/opt/axon/libaxon_pjrt.so')
    bum.upload_artifacts = lambda tmpdir: "local://skipped"
    nc, sched, arrays = _get_compiled(inputs)
    maps = _in_maps(inputs, arrays)
    res = run_bass_kernel_spmd(nc, maps, list(range(NCORES)), trace=True)
    return np.ascontiguousarray(res.results[0]["out"].T), res.exec_time_ns
